# revision 11
# baseline (speedup 1.0000x reference)
"""CT-LSTM (Neural-Hawkes continuous-time LSTM) Trainium2 kernel.

Problem: h_seq[T,B,H] from x[B,T,H], dt[B,T], W[2H,7H], b[7H].
  z = [x_t, h] @ W + b ; 7 gates; c/cbar update; exp decay toward cbar.

The device math runs in ~ms; the wall clock is dominated by the ~40 MB/s
axon tunnel. So the kernel is organized around minimizing tunnel bytes:
  * x ships as fp16 [B*T, H] in natural layout (64 MB); the lhsT tiles for
    the x @ Wx precompute are produced on device with PE transposes.
  * W, bias, and the small identity matrices ship once as a packed fp16
    [128, CW] tensor (8.3 MB), row-sharded across the 8 cores and
    replicated on device with an all_gather (NeuronLink, not the tunnel).
  * h_seq returns as int8 (round(h*127); h in (-1,1) by construction),
    34 MB, decoded to f32 on host.  f32->int8 on ACT/DVE is RNE+saturate.
  * The donated zero output buffers are created on device (jnp.zeros in
    the prep jit), not shipped.
  * All jit callables are built once and cached; per call there is one
    batched device_put, one prep dispatch, one kernel dispatch, one fetch.

Per-core compute (32 batch rows): xz = x @ Wx precomputed into a DRAM
scratch in 64 M-tiles of 128 (b,t) rows; recurrence injects xz_t + b into
PSUM via an [I;ones] identity matmul and accumulates 4 h @ Wh K-tiles on
top; gates via exp/ln activation-table tricks (sigmoid = 1/(1+exp(-z)),
tanh(y) = 1 - 2/(1+exp(2y)), decay = exp(-dt*softplus(zd))).
"""

import numpy as np

B, T, H = 256, 256, 512
NCORES = 8
BL = B // NCORES          # 32 batch rows per core
G = 7 * H                 # 3584 gate columns
KT = H // 128             # 4 K-tiles of each contraction
KTG = KT * G              # 14336
CB = 2 * KTG              # const block base in the packed weight tensor
C_I33 = CB                # [33,32] identity+ones (bias inject lhsT)
C_I32 = CB + 32           # [32,32] identity (h-transpose rhs)
C_I128 = CB + 64          # [128,128] identity (x-transpose rhs)
C_BP = CB + 192           # [1, G] permuted bias (row 0)
CW = CB + 192 + G         # packed width = 32448
F16 = np.float16

# permuted gate order: [gf, gi, gfb, gib, go, gz, gd]
# original z order:    [gi, gf, go, gz, gib, gfb, gd]
_ORIG = {"gi": 0, "gf": 1, "go": 2, "gz": 3, "gib": 4, "gfb": 5, "gd": 6}
_PERM_ORDER = ["gf", "gi", "gfb", "gib", "go", "gz", "gd"]
PERM = np.concatenate(
    [np.arange(_ORIG[g] * H, (_ORIG[g] + 1) * H) for g in _PERM_ORDER]
)
BK = {g: i for i, g in enumerate(_PERM_ORDER)}

_CACHE = {}


def _build(t_steps=T):
    import concourse.bass as bass  # noqa: F401  (side-effect imports)
    import concourse.mybir as mybir
    import concourse.tile as tile
    from concourse import bacc
    from contextlib import ExitStack

    dt_f32 = mybir.dt.float32
    dt_f16 = mybir.dt.float16
    dt_i8 = mybir.dt.int8
    AF = mybir.ActivationFunctionType
    OP = mybir.AluOpType

    nc = bacc.Bacc("TRN2", target_bir_lowering=False, debug=False,
                   enable_asserts=False)
    mt = (BL * t_steps) // 128                       # M-tiles this build

    # ---- DRAM I/O (declaration order == jit parameter order) ----
    xn = nc.dram_tensor("xn", [BL * t_steps, H], dt_f16,
                        kind="ExternalInput").ap()   # natural rows b*T+t
    wg = nc.dram_tensor("wg", [128, CW], dt_f16,
                        kind="ExternalInput").ap()   # wx|wh K-tiles + consts
    dtr = nc.dram_tensor("dtr", [BL, t_steps], dt_f32,
                         kind="ExternalInput").ap()  # raw dt
    out = nc.dram_tensor("out", [t_steps, BL, H], dt_i8,
                         kind="ExternalOutput").ap()

    with tile.TileContext(nc) as tc, ExitStack() as ctx:
        cpool = ctx.enter_context(tc.tile_pool(name="const", bufs=1))
        dpool = ctx.enter_context(tc.tile_pool(name="dram", bufs=1,
                                               space="DRAM"))
        xtp = ctx.enter_context(tc.tile_pool(name="xt", bufs=3))
        xsp = ctx.enter_context(tc.tile_pool(name="xzst", bufs=2))
        gp = ctx.enter_context(tc.tile_pool(name="gates", bufs=1))
        sp_ = ctx.enter_context(tc.tile_pool(name="state", bufs=2))
        hp = ctx.enter_context(tc.tile_pool(name="hout", bufs=2))
        z2p = ctx.enter_context(tc.tile_pool(name="z2", bufs=1, space="PSUM"))
        z1p = ctx.enter_context(tc.tile_pool(name="z1", bufs=3, space="PSUM"))
        # shared pool: x-transpose tiles (precompute) + h-transpose (steps)
        htpp = ctx.enter_context(tc.tile_pool(name="htp", bufs=1,
                                              space="PSUM"))

        # persistent SBUF
        wg_sb = cpool.tile([128, CW], dt_f16, tag="wg")
        nc.sync.dma_start(wg_sb[:], wg[:])
        dt_sb = cpool.tile([BL, t_steps], dt_f32, tag="dt")
        nc.sync.dma_start(dt_sb[:], dtr[:])
        dtneg_sb = cpool.tile([BL, t_steps], dt_f32, tag="dtneg")
        nc.vector.tensor_scalar_mul(dtneg_sb[:], dt_sb[:], -1.0)
        ht_sb = cpool.tile([128, 128], dt_f16, tag="htsb")
        nc.gpsimd.memset(ht_sb[:], 0.0)
        xz_sb0 = cpool.tile([BL + 1, G], dt_f16, tag="xzsb0")
        xz_sb1 = cpool.tile([BL + 1, G], dt_f16, tag="xzsb1")
        xz_sb = [xz_sb0, xz_sb1]
        for i in range(2):
            nc.sync.dma_start(xz_sb[i][BL:BL + 1, :],
                              wg_sb[0:1, C_BP:C_BP + G])

        # warm the activation table set once (exp+ln live together)
        warm = cpool.tile([1, 8], dt_f32, tag="warm")
        nc.gpsimd.memset(warm[:], 1.0)
        nc.scalar.activation(warm[:], warm[:], AF.Exp)
        nc.scalar.activation(warm[:], warm[:], AF.Ln)

        # initial state
        c_prev = sp_.tile([BL, H], dt_f32, tag="c")
        cb_prev = sp_.tile([BL, H], dt_f32, tag="cb")
        nc.gpsimd.memset(c_prev[:], 0.0)
        nc.gpsimd.memset(cb_prev[:], 0.0)

        # xz scratch in device DRAM, [b_loc, t, gate]
        xz_dram = dpool.tile([BL, t_steps, G], dt_f16, tag="xzd")

        def emit_precompute(m):
            """xz rows m*128:(m+1)*128 (= b_loc m//2, t-range (m%2)*128)."""
            b0, t0 = m // 2, (m % 2) * 128
            x_sb = xtp.tile([128, H], dt_f16, tag="xsb")
            nc.sync.dma_start(x_sb[:], xn[m * 128:(m + 1) * 128, :])
            xt_ps = htpp.tile([128, H], dt_f16, tag="tps")
            for k in range(KT):
                nc.tensor.matmul(
                    xt_ps[:, k * 128:(k + 1) * 128],
                    x_sb[:, k * 128:(k + 1) * 128],
                    wg_sb[:, C_I128:C_I128 + 128],
                    is_transpose=True, start=(k == 0), stop=(k == KT - 1))
            xt_sb = xtp.tile([128, H], dt_f16, tag="xtsb")
            nc.scalar.copy(out=xt_sb[:], in_=xt_ps[:])
            for n in range(7):
                ps = z1p.tile([128, 512], dt_f32, tag="z1")
                for k in range(KT):
                    nc.tensor.matmul(
                        ps[:], xt_sb[:, k * 128:(k + 1) * 128],
                        wg_sb[:, k * G + n * 512:k * G + (n + 1) * 512],
                        start=(k == 0), stop=(k == KT - 1))
                xzt = xsp.tile([128, 512], dt_f16, tag="xzstage")
                nc.scalar.copy(out=xzt[:], in_=ps[:])
                nc.sync.dma_start(
                    xz_dram[b0, t0:t0 + 128, n * 512:(n + 1) * 512], xzt[:])

        def sigmoid_into(dst, u_src):
            """dst = 1/(1+u_src) ; u_src = exp(-z) already computed."""
            nc.vector.tensor_scalar_add(u_src[:], u_src[:], 1.0)
            nc.vector.reciprocal_approx_fast(out=dst[:], in_=u_src[:])

        def emit_step(t, c_prev, cb_prev):
            buf = t % 2
            nbuf = (t + 1) % 2
            if t + 1 < t_steps:
                nc.sync.dma_start(xz_sb[nbuf][0:BL, :],
                                  xz_dram[:, t + 1, :])

            # ---- PE: z = xz_t + b + h @ Wh
            z_fi = z2p.tile([BL, 1024], dt_f32, tag="zfi")
            z_fb = z2p.tile([BL, 1024], dt_f32, tag="zfb")
            z_go = z1p.tile([BL, 512], dt_f32, tag="z1")
            z_gz = z1p.tile([BL, 512], dt_f32, tag="z1")
            z_gd = z1p.tile([BL, 512], dt_f32, tag="z1")
            banks = [(z_fi, 0, BK["gf"]), (z_fi, 512, BK["gi"]),
                     (z_gz, 0, BK["gz"]), (z_gd, 0, BK["gd"]),
                     (z_fb, 0, BK["gfb"]), (z_fb, 512, BK["gib"]),
                     (z_go, 0, BK["go"])]
            for (zt, off, bk) in banks:
                dst = zt[:, off:off + 512]
                nc.tensor.matmul(
                    dst, wg_sb[0:BL + 1, C_I33:C_I33 + BL],
                    xz_sb[buf][:, bk * 512:(bk + 1) * 512],
                    start=True, stop=False)
                for k in range(KT):
                    nc.tensor.matmul(
                        dst, ht_sb[:, k * BL:(k + 1) * BL],
                        wg_sb[:, KTG + k * G + bk * 512:
                              KTG + k * G + (bk + 1) * 512],
                        start=False, stop=(k == KT - 1))

            # ---- gates
            u_fi = gp.tile([BL, 1024], dt_f32, tag="ufi")
            nc.scalar.activation(u_fi[:], z_fi[:], AF.Exp, scale=-1.0)
            s_fi = gp.tile([BL, 1024], dt_f32, tag="sfi")
            sigmoid_into(s_fi, u_fi)                       # gf | gi

            u_gz = gp.tile([BL, 512], dt_f32, tag="ugz")
            nc.scalar.activation(u_gz[:], z_gz[:], AF.Exp, scale=-2.0)
            nc.vector.tensor_scalar_add(u_gz[:], u_gz[:], 1.0)
            t_z = gp.tile([BL, 512], dt_f32, tag="tz")
            nc.vector.reciprocal_approx_fast(out=t_z[:], in_=u_gz[:])
            nc.vector.tensor_scalar(t_z[:], t_z[:], -2.0, 1.0,
                                    OP.mult, OP.add)       # tanh(gz)

            # decay: E = exp(-dt * softplus(zd))
            u_gd = gp.tile([BL, 512], dt_f32, tag="ugd")
            nc.scalar.activation(u_gd[:], z_gd[:], AF.Exp)
            nc.gpsimd.tensor_scalar_add(u_gd[:], u_gd[:], 1.0)
            sp_t = gp.tile([BL, 512], dt_f32, tag="spt")
            nc.scalar.activation(sp_t[:], u_gd[:], AF.Ln)
            e_t = gp.tile([BL, 512], dt_f32, tag="et")
            nc.scalar.activation(e_t[:], sp_t[:], AF.Exp,
                                 scale=dtneg_sb[:, t:t + 1])

            u_fb = gp.tile([BL, 1024], dt_f32, tag="ufb")
            nc.scalar.activation(u_fb[:], z_fb[:], AF.Exp, scale=-1.0)
            s_fb = gp.tile([BL, 1024], dt_f32, tag="sfb")
            sigmoid_into(s_fb, u_fb)                       # gfb | gib

            u_go = gp.tile([BL, 512], dt_f32, tag="ugo")
            nc.scalar.activation(u_go[:], z_go[:], AF.Exp, scale=-1.0)
            nc.gpsimd.tensor_scalar_add(u_go[:], u_go[:], 1.0)
            s_go = gp.tile([BL, 512], dt_f32, tag="sgo")
            nc.vector.reciprocal_approx_fast(out=s_go[:], in_=u_go[:])

            # ---- state update
            p1 = gp.tile([BL, 512], dt_f32, tag="p1")
            nc.vector.tensor_mul(p1[:], s_fi[:, 0:512], c_prev[:])
            p2 = gp.tile([BL, 512], dt_f32, tag="p2")
            nc.gpsimd.tensor_mul(p2[:], s_fi[:, 512:1024], t_z[:])
            c_new = gp.tile([BL, H], dt_f32, tag="cn")
            nc.vector.tensor_add(c_new[:], p1[:], p2[:])

            q1 = gp.tile([BL, 512], dt_f32, tag="q1")
            nc.vector.tensor_mul(q1[:], s_fb[:, 0:512], cb_prev[:])
            q2 = gp.tile([BL, 512], dt_f32, tag="q2")
            nc.gpsimd.tensor_mul(q2[:], s_fb[:, 512:1024], t_z[:])
            cb_new = sp_.tile([BL, H], dt_f32, tag="cb")
            nc.vector.tensor_add(cb_new[:], q1[:], q2[:])

            w_t = gp.tile([BL, 512], dt_f32, tag="wt")
            nc.vector.tensor_sub(w_t[:], c_new[:], cb_new[:])
            nc.vector.tensor_mul(w_t[:], w_t[:], e_t[:])
            cd = sp_.tile([BL, H], dt_f32, tag="c")   # carried decayed cell
            nc.vector.tensor_add(cd[:], cb_new[:], w_t[:])

            # h = go * tanh(cd)
            u_c = gp.tile([BL, 512], dt_f32, tag="uc")
            nc.scalar.activation(u_c[:], cd[:], AF.Exp, scale=-2.0)
            nc.vector.tensor_scalar_add(u_c[:], u_c[:], 1.0)
            t_c = gp.tile([BL, 512], dt_f32, tag="tc")
            nc.vector.reciprocal_approx_fast(out=t_c[:], in_=u_c[:])
            nc.vector.tensor_scalar(t_c[:], t_c[:], -2.0, 1.0,
                                    OP.mult, OP.add)
            h16 = hp.tile([BL, H], dt_f16, tag="h16")
            nc.vector.tensor_mul(h16[:], s_go[:], t_c[:])
            oi = hp.tile([BL, H], dt_i8, tag="oi")
            nc.vector.tensor_scalar_mul(oi[:], h16[:], 127.0)
            nc.sync.dma_start(out[t], oi[:])

            # hT for next step: 4 PE transposes, one group, then one copy
            if t + 1 < t_steps:
                htp = htpp.tile([128, H], dt_f16, tag="tps")
                for k in range(KT):
                    nc.tensor.matmul(
                        htp[:, k * BL:(k + 1) * BL],
                        h16[:, k * 128:(k + 1) * 128],
                        wg_sb[0:BL, C_I32:C_I32 + BL],
                        is_transpose=True,
                        start=(k == 0), stop=(k == KT - 1))
                nc.scalar.copy(out=ht_sb[:], in_=htp[:, 0:128])

            return cd, cb_new

        # xz precompute: even M-tiles cover t<128 for every b, odd the rest.
        for m in range(0, mt, 2):
            emit_precompute(m)
        for m in range(1, mt, 2):
            emit_precompute(m)
        nc.sync.dma_start(xz_sb[0][0:BL, :], xz_dram[:, 0, :])
        for t in range(t_steps):
            c_prev, cb_prev = emit_step(t, c_prev, cb_prev)

    nc.compile()
    return nc


def _get_rt(t_steps=T):
    """Build-once runtime: bass module + cached jit callables."""
    key = ("rt", t_steps)
    if key in _CACHE:
        return _CACHE[key]
    import jax
    import jax.numpy as jnp
    import concourse.mybir as mybir
    from concourse import bass2jax
    from concourse.bass2jax import shard_map, Mesh, PartitionSpec
    from jax.sharding import NamedSharding
    from types import SimpleNamespace

    bass2jax.install_neuronx_cc_hook()
    nc = _build(t_steps)

    partition_name = (nc.partition_id_tensor.name
                      if nc.partition_id_tensor else None)
    in_names, out_names, out_avals = [], [], []
    for alloc in nc.m.functions[0].allocations:
        if not isinstance(alloc, mybir.MemoryLocationSet):
            continue
        name = alloc.memorylocations[0].name
        if alloc.kind == "ExternalInput":
            if name != partition_name:
                in_names.append(name)
        elif alloc.kind == "ExternalOutput":
            out_names.append(name)
            shape = tuple(alloc.tensor_shape)
            out_avals.append(
                jax.core.ShapedArray(shape, mybir.dt.np(alloc.dtype)))
    n_params = len(in_names)
    all_names = in_names + out_names
    if partition_name is not None:
        all_names = all_names + [partition_name]
    all_names = tuple(all_names)
    donate = tuple(range(n_params, n_params + len(out_names)))

    def _body(*args):
        operands = list(args)
        if partition_name is not None:
            operands.append(bass2jax.partition_id_tensor())
        outs = bass2jax._bass_exec_p.bind(
            *operands, out_avals=tuple(out_avals), in_names=all_names,
            out_names=tuple(out_names), lowering_input_output_aliases=(),
            sim_require_finite=True, sim_require_nnan=True, nc=nc)
        return tuple(outs)

    devs = jax.devices()[:NCORES]
    mesh = Mesh(np.asarray(devs), ("core",))
    P = PartitionSpec
    # out assembled as [T, NCORES*BL, H] == [T, B, H]: no host transpose.
    # The donated zero buffers use the same sharding so aliasing is clean.
    in_specs = (P("core"),) * n_params + (P(None, "core"),) * len(out_names)
    sharded = jax.jit(
        shard_map(_body, mesh=mesh, in_specs=in_specs,
                  out_specs=(P(None, "core"),) * len(out_names),
                  check_rep=False),
        donate_argnums=donate, keep_unused=True)

    def _prep_body(w):
        wr = jax.lax.all_gather(w, "core", axis=0, tiled=True)
        z = jnp.zeros((t_steps, BL, H), jnp.int8)
        return wr, z

    prep = jax.jit(
        shard_map(_prep_body, mesh=mesh, in_specs=(P("core"),),
                  out_specs=(P("core"), P(None, "core")), check_rep=False))

    rt = SimpleNamespace(
        nc=nc, sharded=sharded, prep=prep, in_names=in_names,
        shc=NamedSharding(mesh, P("core")), jax=jax)
    _CACHE[key] = rt
    return rt


def _pack_wg(W, b):
    Wp = np.asarray(W, np.float32)[:, PERM].astype(F16)
    wx, wh = Wp[:H], Wp[H:]
    wg = np.zeros((128, CW), F16)
    for k in range(KT):
        wg[:, k * G:(k + 1) * G] = wx[k * 128:(k + 1) * 128]
        wg[:, KTG + k * G:KTG + (k + 1) * G] = wh[k * 128:(k + 1) * 128]
    wg[0:BL, C_I33:C_I33 + BL] = np.eye(BL, dtype=F16)
    wg[BL, C_I33:C_I33 + BL] = 1.0                      # bias inject row
    wg[0:BL, C_I32:C_I32 + BL] = np.eye(BL, dtype=F16)
    wg[0:128, C_I128:C_I128 + 128] = np.eye(128, dtype=F16)
    wg[0, C_BP:C_BP + G] = np.asarray(b, np.float32)[PERM].astype(F16)
    return wg


def kernel(x, dt, W, b):
    rt = _get_rt(T)
    jax = rt.jax

    wg = _pack_wg(W, b)
    dtg = np.ascontiguousarray(np.asarray(dt, np.float32))
    xg = np.asarray(x, np.float32).reshape(B * T, H).astype(F16)

    # wg/dtg first: prep's all_gather+zeros overlap the big x upload
    wd, dtd, xd = jax.device_put((wg, dtg, xg), (rt.shc, rt.shc, rt.shc))
    wr, zz = rt.prep(wd)
    args = {"xn": xd, "wg": wr, "dtr": dtd}
    (outg,) = rt.sharded(*[args[n] for n in rt.in_names], zz)
    o = np.asarray(outg)                       # [T, B, H] int8
    return np.multiply(o, np.float32(1.0 / 127.0), dtype=np.float32)


# revision 12
# speedup vs baseline: 1.2620x; 1.2620x over previous
"""CT-LSTM (Neural-Hawkes continuous-time LSTM) Trainium2 kernel.

Problem: h_seq[T,B,H] from x[B,T,H], dt[B,T], W[2H,7H], b[7H].
  z = [x_t, h] @ W + b ; 7 gates; c/cbar update; exp decay toward cbar.

The device math runs in ~ms; the wall clock is dominated by the ~40 MB/s
axon tunnel. So the kernel is organized around minimizing tunnel bytes:
  * x ships as fp16 [B*T, H] in natural layout (64 MB); the lhsT tiles for
    the x @ Wx precompute are produced on device with PE transposes.
  * W, bias, and the small identity matrices ship once as a packed fp16
    [128, CW] tensor (8.3 MB), row-sharded across the 8 cores and
    replicated on device with an all_gather (NeuronLink, not the tunnel).
  * h_seq returns as int8 (round(h*127); h in (-1,1) by construction),
    34 MB, decoded to f32 on host.  f32->int8 on ACT/DVE is RNE+saturate.
  * The donated zero output buffers are created on device (jnp.zeros in
    the prep jit), not shipped.
  * All jit callables are built once and cached; per call there is one
    batched device_put, one prep dispatch, one kernel dispatch, one fetch.

Per-core compute (32 batch rows): xz = x @ Wx precomputed into a DRAM
scratch in 64 M-tiles of 128 (b,t) rows; recurrence injects xz_t + b into
PSUM via an [I;ones] identity matmul and accumulates 4 h @ Wh K-tiles on
top; gates via exp/ln activation-table tricks (sigmoid = 1/(1+exp(-z)),
tanh(y) = 1 - 2/(1+exp(2y)), decay = exp(-dt*softplus(zd))).
"""

import numpy as np

B, T, H = 256, 256, 512
NCORES = 8
BL = B // NCORES          # 32 batch rows per core
G = 7 * H                 # 3584 gate columns
KT = H // 128             # 4 K-tiles of each contraction
KTG = KT * G              # 14336
CB = 2 * KTG              # const block base in the packed weight tensor
C_I33 = CB                # [33,32] identity+ones (bias inject lhsT)
C_I32 = CB + 32           # [32,32] identity (h-transpose rhs)
C_I128 = CB + 64          # [128,128] identity (x-transpose rhs)
C_BP = CB + 192           # [1, G] permuted bias (row 0)
CW = CB + 192 + G         # packed width = 32448
F16 = np.float16

# permuted gate order: [gf, gi, gfb, gib, go, gz, gd]
# original z order:    [gi, gf, go, gz, gib, gfb, gd]
_ORIG = {"gi": 0, "gf": 1, "go": 2, "gz": 3, "gib": 4, "gfb": 5, "gd": 6}
_PERM_ORDER = ["gf", "gi", "gfb", "gib", "go", "gz", "gd"]
PERM = np.concatenate(
    [np.arange(_ORIG[g] * H, (_ORIG[g] + 1) * H) for g in _PERM_ORDER]
)
BK = {g: i for i, g in enumerate(_PERM_ORDER)}

_CACHE = {}


def _build(t_steps=T):
    import concourse.bass as bass  # noqa: F401  (side-effect imports)
    import concourse.mybir as mybir
    import concourse.tile as tile
    from concourse import bacc
    from contextlib import ExitStack

    dt_f32 = mybir.dt.float32
    dt_f16 = mybir.dt.float16
    dt_i8 = mybir.dt.int8
    AF = mybir.ActivationFunctionType
    OP = mybir.AluOpType

    nc = bacc.Bacc("TRN2", target_bir_lowering=False, debug=False,
                   enable_asserts=False)
    mt = (BL * t_steps) // 128                       # M-tiles this build

    # ---- DRAM I/O (declaration order == jit parameter order) ----
    xn = nc.dram_tensor("xn", [BL * t_steps, H], dt_f16,
                        kind="ExternalInput").ap()   # natural rows b*T+t
    wg = nc.dram_tensor("wg", [128, CW], dt_f16,
                        kind="ExternalInput").ap()   # wx|wh K-tiles + consts
    dtr = nc.dram_tensor("dtr", [BL, t_steps], dt_f32,
                         kind="ExternalInput").ap()  # raw dt
    out = nc.dram_tensor("out", [t_steps, BL, H], dt_i8,
                         kind="ExternalOutput").ap()

    with tile.TileContext(nc) as tc, ExitStack() as ctx:
        cpool = ctx.enter_context(tc.tile_pool(name="const", bufs=1))
        dpool = ctx.enter_context(tc.tile_pool(name="dram", bufs=1,
                                               space="DRAM"))
        xtp = ctx.enter_context(tc.tile_pool(name="xt", bufs=3))
        xsp = ctx.enter_context(tc.tile_pool(name="xzst", bufs=2))
        gp = ctx.enter_context(tc.tile_pool(name="gates", bufs=1))
        sp_ = ctx.enter_context(tc.tile_pool(name="state", bufs=2))
        hp = ctx.enter_context(tc.tile_pool(name="hout", bufs=2))
        z2p = ctx.enter_context(tc.tile_pool(name="z2", bufs=1, space="PSUM"))
        z1p = ctx.enter_context(tc.tile_pool(name="z1", bufs=3, space="PSUM"))
        # shared pool: x-transpose tiles (precompute) + h-transpose (steps)
        htpp = ctx.enter_context(tc.tile_pool(name="htp", bufs=1,
                                              space="PSUM"))

        # persistent SBUF
        wg_sb = cpool.tile([128, CW], dt_f16, tag="wg")
        nc.sync.dma_start(wg_sb[:], wg[:])
        dt_sb = cpool.tile([BL, t_steps], dt_f32, tag="dt")
        nc.sync.dma_start(dt_sb[:], dtr[:])
        dtneg_sb = cpool.tile([BL, t_steps], dt_f32, tag="dtneg")
        nc.vector.tensor_scalar_mul(dtneg_sb[:], dt_sb[:], -1.0)
        ht_sb = cpool.tile([128, 128], dt_f16, tag="htsb")
        nc.gpsimd.memset(ht_sb[:], 0.0)
        xz_sb0 = cpool.tile([BL + 1, G], dt_f16, tag="xzsb0")
        xz_sb1 = cpool.tile([BL + 1, G], dt_f16, tag="xzsb1")
        xz_sb = [xz_sb0, xz_sb1]
        for i in range(2):
            nc.sync.dma_start(xz_sb[i][BL:BL + 1, :],
                              wg_sb[0:1, C_BP:C_BP + G])

        # warm the activation table set once (exp+ln live together)
        warm = cpool.tile([1, 8], dt_f32, tag="warm")
        nc.gpsimd.memset(warm[:], 1.0)
        nc.scalar.activation(warm[:], warm[:], AF.Exp)
        nc.scalar.activation(warm[:], warm[:], AF.Ln)

        # initial state
        c_prev = sp_.tile([BL, H], dt_f32, tag="c")
        cb_prev = sp_.tile([BL, H], dt_f32, tag="cb")
        nc.gpsimd.memset(c_prev[:], 0.0)
        nc.gpsimd.memset(cb_prev[:], 0.0)

        # xz scratch in device DRAM, [b_loc, t, gate]
        xz_dram = dpool.tile([BL, t_steps, G], dt_f16, tag="xzd")

        def emit_precompute(m):
            """xz rows m*128:(m+1)*128 (= b_loc m//2, t-range (m%2)*128)."""
            b0, t0 = m // 2, (m % 2) * 128
            x_sb = xtp.tile([128, H], dt_f16, tag="xsb")
            nc.sync.dma_start(x_sb[:], xn[m * 128:(m + 1) * 128, :])
            xt_ps = htpp.tile([128, H], dt_f16, tag="tps")
            for k in range(KT):
                nc.tensor.matmul(
                    xt_ps[:, k * 128:(k + 1) * 128],
                    x_sb[:, k * 128:(k + 1) * 128],
                    wg_sb[:, C_I128:C_I128 + 128],
                    is_transpose=True, start=(k == 0), stop=(k == KT - 1))
            xt_sb = xtp.tile([128, H], dt_f16, tag="xtsb")
            nc.scalar.copy(out=xt_sb[:], in_=xt_ps[:])
            for n in range(7):
                ps = z1p.tile([128, 512], dt_f32, tag="z1")
                for k in range(KT):
                    nc.tensor.matmul(
                        ps[:], xt_sb[:, k * 128:(k + 1) * 128],
                        wg_sb[:, k * G + n * 512:k * G + (n + 1) * 512],
                        start=(k == 0), stop=(k == KT - 1))
                xzt = xsp.tile([128, 512], dt_f16, tag="xzstage")
                nc.scalar.copy(out=xzt[:], in_=ps[:])
                nc.sync.dma_start(
                    xz_dram[b0, t0:t0 + 128, n * 512:(n + 1) * 512], xzt[:])

        def sigmoid_into(dst, u_src):
            """dst = 1/(1+u_src) ; u_src = exp(-z) already computed."""
            nc.vector.tensor_scalar_add(u_src[:], u_src[:], 1.0)
            nc.vector.reciprocal_approx_fast(out=dst[:], in_=u_src[:])

        def emit_step(t, c_prev, cb_prev):
            buf = t % 2
            nbuf = (t + 1) % 2
            if t + 1 < t_steps:
                nc.sync.dma_start(xz_sb[nbuf][0:BL, :],
                                  xz_dram[:, t + 1, :])

            # ---- PE: z = xz_t + b + h @ Wh
            z_fi = z2p.tile([BL, 1024], dt_f32, tag="zfi")
            z_fb = z2p.tile([BL, 1024], dt_f32, tag="zfb")
            z_go = z1p.tile([BL, 512], dt_f32, tag="z1")
            z_gz = z1p.tile([BL, 512], dt_f32, tag="z1")
            z_gd = z1p.tile([BL, 512], dt_f32, tag="z1")
            banks = [(z_fi, 0, BK["gf"]), (z_fi, 512, BK["gi"]),
                     (z_gz, 0, BK["gz"]), (z_gd, 0, BK["gd"]),
                     (z_fb, 0, BK["gfb"]), (z_fb, 512, BK["gib"]),
                     (z_go, 0, BK["go"])]
            for (zt, off, bk) in banks:
                dst = zt[:, off:off + 512]
                nc.tensor.matmul(
                    dst, wg_sb[0:BL + 1, C_I33:C_I33 + BL],
                    xz_sb[buf][:, bk * 512:(bk + 1) * 512],
                    start=True, stop=False)
                for k in range(KT):
                    nc.tensor.matmul(
                        dst, ht_sb[:, k * BL:(k + 1) * BL],
                        wg_sb[:, KTG + k * G + bk * 512:
                              KTG + k * G + (bk + 1) * 512],
                        start=False, stop=(k == KT - 1))

            # ---- gates
            u_fi = gp.tile([BL, 1024], dt_f32, tag="ufi")
            nc.scalar.activation(u_fi[:], z_fi[:], AF.Exp, scale=-1.0)
            s_fi = gp.tile([BL, 1024], dt_f32, tag="sfi")
            sigmoid_into(s_fi, u_fi)                       # gf | gi

            u_gz = gp.tile([BL, 512], dt_f32, tag="ugz")
            nc.scalar.activation(u_gz[:], z_gz[:], AF.Exp, scale=-2.0)
            nc.vector.tensor_scalar_add(u_gz[:], u_gz[:], 1.0)
            t_z = gp.tile([BL, 512], dt_f32, tag="tz")
            nc.vector.reciprocal_approx_fast(out=t_z[:], in_=u_gz[:])
            nc.vector.tensor_scalar(t_z[:], t_z[:], -2.0, 1.0,
                                    OP.mult, OP.add)       # tanh(gz)

            # decay: E = exp(-dt * softplus(zd))
            u_gd = gp.tile([BL, 512], dt_f32, tag="ugd")
            nc.scalar.activation(u_gd[:], z_gd[:], AF.Exp)
            nc.gpsimd.tensor_scalar_add(u_gd[:], u_gd[:], 1.0)
            sp_t = gp.tile([BL, 512], dt_f32, tag="spt")
            nc.scalar.activation(sp_t[:], u_gd[:], AF.Ln)
            e_t = gp.tile([BL, 512], dt_f32, tag="et")
            nc.scalar.activation(e_t[:], sp_t[:], AF.Exp,
                                 scale=dtneg_sb[:, t:t + 1])

            u_fb = gp.tile([BL, 1024], dt_f32, tag="ufb")
            nc.scalar.activation(u_fb[:], z_fb[:], AF.Exp, scale=-1.0)
            s_fb = gp.tile([BL, 1024], dt_f32, tag="sfb")
            sigmoid_into(s_fb, u_fb)                       # gfb | gib

            u_go = gp.tile([BL, 512], dt_f32, tag="ugo")
            nc.scalar.activation(u_go[:], z_go[:], AF.Exp, scale=-1.0)
            nc.gpsimd.tensor_scalar_add(u_go[:], u_go[:], 1.0)
            s_go = gp.tile([BL, 512], dt_f32, tag="sgo")
            nc.vector.reciprocal_approx_fast(out=s_go[:], in_=u_go[:])

            # ---- state update
            p1 = gp.tile([BL, 512], dt_f32, tag="p1")
            nc.vector.tensor_mul(p1[:], s_fi[:, 0:512], c_prev[:])
            p2 = gp.tile([BL, 512], dt_f32, tag="p2")
            nc.gpsimd.tensor_mul(p2[:], s_fi[:, 512:1024], t_z[:])
            c_new = gp.tile([BL, H], dt_f32, tag="cn")
            nc.vector.tensor_add(c_new[:], p1[:], p2[:])

            q1 = gp.tile([BL, 512], dt_f32, tag="q1")
            nc.vector.tensor_mul(q1[:], s_fb[:, 0:512], cb_prev[:])
            q2 = gp.tile([BL, 512], dt_f32, tag="q2")
            nc.gpsimd.tensor_mul(q2[:], s_fb[:, 512:1024], t_z[:])
            cb_new = sp_.tile([BL, H], dt_f32, tag="cb")
            nc.vector.tensor_add(cb_new[:], q1[:], q2[:])

            w_t = gp.tile([BL, 512], dt_f32, tag="wt")
            nc.vector.tensor_sub(w_t[:], c_new[:], cb_new[:])
            nc.vector.tensor_mul(w_t[:], w_t[:], e_t[:])
            cd = sp_.tile([BL, H], dt_f32, tag="c")   # carried decayed cell
            nc.vector.tensor_add(cd[:], cb_new[:], w_t[:])

            # h = go * tanh(cd)
            u_c = gp.tile([BL, 512], dt_f32, tag="uc")
            nc.scalar.activation(u_c[:], cd[:], AF.Exp, scale=-2.0)
            nc.vector.tensor_scalar_add(u_c[:], u_c[:], 1.0)
            t_c = gp.tile([BL, 512], dt_f32, tag="tc")
            nc.vector.reciprocal_approx_fast(out=t_c[:], in_=u_c[:])
            nc.vector.tensor_scalar(t_c[:], t_c[:], -2.0, 1.0,
                                    OP.mult, OP.add)
            h16 = hp.tile([BL, H], dt_f16, tag="h16")
            nc.vector.tensor_mul(h16[:], s_go[:], t_c[:])
            oi = hp.tile([BL, H], dt_i8, tag="oi")
            nc.vector.tensor_scalar_mul(oi[:], h16[:], 127.0)
            nc.sync.dma_start(out[t], oi[:])

            # hT for next step: 4 PE transposes, one group, then one copy
            if t + 1 < t_steps:
                htp = htpp.tile([128, H], dt_f16, tag="tps")
                for k in range(KT):
                    nc.tensor.matmul(
                        htp[:, k * BL:(k + 1) * BL],
                        h16[:, k * 128:(k + 1) * 128],
                        wg_sb[0:BL, C_I32:C_I32 + BL],
                        is_transpose=True,
                        start=(k == 0), stop=(k == KT - 1))
                nc.scalar.copy(out=ht_sb[:], in_=htp[:, 0:128])

            return cd, cb_new

        # xz precompute: even M-tiles cover t<128 for every b, odd the rest.
        for m in range(0, mt, 2):
            emit_precompute(m)
        for m in range(1, mt, 2):
            emit_precompute(m)
        nc.sync.dma_start(xz_sb[0][0:BL, :], xz_dram[:, 0, :])
        for t in range(t_steps):
            c_prev, cb_prev = emit_step(t, c_prev, cb_prev)

    nc.compile()
    return nc


def _get_rt(t_steps=T):
    """Build-once runtime: bass module + cached jit callables."""
    key = ("rt", t_steps)
    if key in _CACHE:
        return _CACHE[key]
    import jax
    import jax.numpy as jnp
    import concourse.mybir as mybir
    from concourse import bass2jax
    from concourse.bass2jax import shard_map, Mesh, PartitionSpec
    from jax.sharding import NamedSharding
    from types import SimpleNamespace

    bass2jax.install_neuronx_cc_hook()
    nc = _build(t_steps)

    partition_name = (nc.partition_id_tensor.name
                      if nc.partition_id_tensor else None)
    in_names, out_names, out_avals = [], [], []
    for alloc in nc.m.functions[0].allocations:
        if not isinstance(alloc, mybir.MemoryLocationSet):
            continue
        name = alloc.memorylocations[0].name
        if alloc.kind == "ExternalInput":
            if name != partition_name:
                in_names.append(name)
        elif alloc.kind == "ExternalOutput":
            out_names.append(name)
            shape = tuple(alloc.tensor_shape)
            out_avals.append(
                jax.core.ShapedArray(shape, mybir.dt.np(alloc.dtype)))
    n_params = len(in_names)
    all_names = in_names + out_names
    if partition_name is not None:
        all_names = all_names + [partition_name]
    all_names = tuple(all_names)
    donate = tuple(range(n_params, n_params + len(out_names)))

    def _body(*args):
        operands = list(args)
        if partition_name is not None:
            operands.append(bass2jax.partition_id_tensor())
        outs = bass2jax._bass_exec_p.bind(
            *operands, out_avals=tuple(out_avals), in_names=all_names,
            out_names=tuple(out_names), lowering_input_output_aliases=(),
            sim_require_finite=True, sim_require_nnan=True, nc=nc)
        return tuple(outs)

    devs = jax.devices()[:NCORES]
    mesh = Mesh(np.asarray(devs), ("core",))
    P = PartitionSpec
    # out assembled as [T, NCORES*BL, H] == [T, B, H]: no host transpose.
    # The donated zero buffers use the same sharding so aliasing is clean.
    in_specs = (P("core"),) * n_params + (P(None, "core"),) * len(out_names)
    sharded = jax.jit(
        shard_map(_body, mesh=mesh, in_specs=in_specs,
                  out_specs=(P(None, "core"),) * len(out_names),
                  check_rep=False),
        donate_argnums=donate, keep_unused=True)

    def _prep_body(w):
        wr = jax.lax.all_gather(w, "core", axis=0, tiled=True)
        z = jnp.zeros((t_steps, BL, H), jnp.int8)
        return wr, z

    prep = jax.jit(
        shard_map(_prep_body, mesh=mesh, in_specs=(P("core"),),
                  out_specs=(P("core"), P(None, "core")), check_rep=False))

    rt = SimpleNamespace(
        nc=nc, sharded=sharded, prep=prep, in_names=in_names,
        shc=NamedSharding(mesh, P("core")), jax=jax)
    _CACHE[key] = rt
    return rt


def _pack_wg(W, b):
    Wp = np.asarray(W, np.float32)[:, PERM].astype(F16)
    wx, wh = Wp[:H], Wp[H:]
    wg = np.zeros((128, CW), F16)
    for k in range(KT):
        wg[:, k * G:(k + 1) * G] = wx[k * 128:(k + 1) * 128]
        wg[:, KTG + k * G:KTG + (k + 1) * G] = wh[k * 128:(k + 1) * 128]
    wg[0:BL, C_I33:C_I33 + BL] = np.eye(BL, dtype=F16)
    wg[BL, C_I33:C_I33 + BL] = 1.0                      # bias inject row
    wg[0:BL, C_I32:C_I32 + BL] = np.eye(BL, dtype=F16)
    wg[0:128, C_I128:C_I128 + 128] = np.eye(128, dtype=F16)
    wg[0, C_BP:C_BP + G] = np.asarray(b, np.float32)[PERM].astype(F16)
    return wg


def kernel(x, dt, W, b):
    import os
    import time
    dbg = os.environ.get("KERNEL_DEBUG_TIMING")
    t0 = time.time()
    rt = _get_rt(T)
    jax = rt.jax

    wg = _pack_wg(W, b)
    dtg = np.ascontiguousarray(np.asarray(dt, np.float32))
    xg = np.asarray(x, np.float32).reshape(B * T, H).astype(F16)
    t1 = time.time()

    # wg/dtg first: prep's all_gather+zeros overlap the big x upload
    wd, dtd, xd = jax.device_put((wg, dtg, xg), (rt.shc, rt.shc, rt.shc))
    wr, zz = rt.prep(wd)
    args = {"xn": xd, "wg": wr, "dtr": dtd}
    (outg,) = rt.sharded(*[args[n] for n in rt.in_names], zz)
    t2 = time.time()
    o = np.asarray(outg)                       # [T, B, H] int8
    t3 = time.time()
    res = np.multiply(o, np.float32(1.0 / 127.0), dtype=np.float32)
    if dbg:
        print(f"  [kernel] hostprep={t1-t0:.2f} dispatch={t2-t1:.2f} "
              f"fetch={t3-t2:.2f} decode={time.time()-t3:.2f}", flush=True)
    return res


# revision 13
# speedup vs baseline: 1.6203x; 1.2839x over previous
"""CT-LSTM (Neural-Hawkes continuous-time LSTM) Trainium2 kernel.

Problem: h_seq[T,B,H] from x[B,T,H], dt[B,T], W[2H,7H], b[7H].
  z = [x_t, h] @ W + b ; 7 gates; c/cbar update; exp decay toward cbar.

The device math runs in ~ms; the wall clock is dominated by the ~40 MB/s
axon tunnel. So the kernel is organized around minimizing tunnel bytes:
  * x ships as fp16 [B*T, H] in natural layout (64 MB); the lhsT tiles for
    the x @ Wx precompute are produced on device with PE transposes.
  * W, bias, and the small identity matrices ship once as a packed fp16
    [128, CW] tensor (8.3 MB), row-sharded across the 8 cores and
    replicated on device with an all_gather (NeuronLink, not the tunnel).
  * h_seq returns as int8 (round(h*127); h in (-1,1) by construction),
    34 MB, decoded to f32 on host.  f32->int8 on ACT/DVE is RNE+saturate.
  * The donated zero output buffers are created on device (jnp.zeros in
    the prep jit), not shipped.
  * All jit callables are built once and cached; per call there is one
    batched device_put, one prep dispatch, one kernel dispatch, one fetch.

Per-core compute (32 batch rows): xz = x @ Wx precomputed into a DRAM
scratch in 64 M-tiles of 128 (b,t) rows; recurrence injects xz_t + b into
PSUM via an [I;ones] identity matmul and accumulates 4 h @ Wh K-tiles on
top; gates via exp/ln activation-table tricks (sigmoid = 1/(1+exp(-z)),
tanh(y) = 1 - 2/(1+exp(2y)), decay = exp(-dt*softplus(zd))).
"""

import numpy as np

B, T, H = 256, 256, 512
NCORES = 8
BL = B // NCORES          # 32 batch rows per core
G = 7 * H                 # 3584 gate columns
KT = H // 128             # 4 K-tiles of each contraction
KTG = KT * G              # 14336
CB = 2 * KTG              # const block base in the packed weight tensor
C_I33 = CB                # [33,32] identity+ones (bias inject lhsT)
C_I32 = CB + 32           # [32,32] identity (h-transpose rhs)
C_I128 = CB + 64          # [128,128] identity (x-transpose rhs)
C_BP = CB + 192           # [1, G] permuted bias (row 0)
CW = CB + 192 + G         # packed width = 32448
F16 = np.float16

# permuted gate order: [gf, gi, gfb, gib, go, gz, gd]
# original z order:    [gi, gf, go, gz, gib, gfb, gd]
_ORIG = {"gi": 0, "gf": 1, "go": 2, "gz": 3, "gib": 4, "gfb": 5, "gd": 6}
_PERM_ORDER = ["gf", "gi", "gfb", "gib", "go", "gz", "gd"]
PERM = np.concatenate(
    [np.arange(_ORIG[g] * H, (_ORIG[g] + 1) * H) for g in _PERM_ORDER]
)
BK = {g: i for i, g in enumerate(_PERM_ORDER)}

_CACHE = {}


def _build(t_steps=T):
    import concourse.bass as bass  # noqa: F401  (side-effect imports)
    import concourse.mybir as mybir
    import concourse.tile as tile
    from concourse import bacc
    from contextlib import ExitStack

    dt_f32 = mybir.dt.float32
    dt_f16 = mybir.dt.float16
    dt_i8 = mybir.dt.int8
    AF = mybir.ActivationFunctionType
    OP = mybir.AluOpType

    nc = bacc.Bacc("TRN2", target_bir_lowering=False, debug=False,
                   enable_asserts=False)
    mt = (BL * t_steps) // 128                       # M-tiles this build

    # ---- DRAM I/O (declaration order == jit parameter order) ----
    xn = nc.dram_tensor("xn", [BL * t_steps, H], dt_f16,
                        kind="ExternalInput").ap()   # natural rows b*T+t
    wg = nc.dram_tensor("wg", [128, CW], dt_f16,
                        kind="ExternalInput").ap()   # wx|wh K-tiles + consts
    dtr = nc.dram_tensor("dtr", [BL, t_steps], dt_f32,
                         kind="ExternalInput").ap()  # raw dt
    out = nc.dram_tensor("out", [t_steps, BL, H], dt_i8,
                         kind="ExternalOutput").ap()

    with tile.TileContext(nc) as tc, ExitStack() as ctx:
        cpool = ctx.enter_context(tc.tile_pool(name="const", bufs=1))
        dpool = ctx.enter_context(tc.tile_pool(name="dram", bufs=1,
                                               space="DRAM"))
        xtp = ctx.enter_context(tc.tile_pool(name="xt", bufs=3))
        xsp = ctx.enter_context(tc.tile_pool(name="xzst", bufs=2))
        gp = ctx.enter_context(tc.tile_pool(name="gates", bufs=1))
        sp_ = ctx.enter_context(tc.tile_pool(name="state", bufs=2))
        hp = ctx.enter_context(tc.tile_pool(name="hout", bufs=2))
        z2p = ctx.enter_context(tc.tile_pool(name="z2", bufs=1, space="PSUM"))
        z1p = ctx.enter_context(tc.tile_pool(name="z1", bufs=3, space="PSUM"))
        # shared pool: x-transpose tiles (precompute) + h-transpose (steps)
        htpp = ctx.enter_context(tc.tile_pool(name="htp", bufs=1,
                                              space="PSUM"))

        # persistent SBUF
        wg_sb = cpool.tile([128, CW], dt_f16, tag="wg")
        nc.sync.dma_start(wg_sb[:], wg[:])
        dt_sb = cpool.tile([BL, t_steps], dt_f32, tag="dt")
        nc.sync.dma_start(dt_sb[:], dtr[:])
        dtneg_sb = cpool.tile([BL, t_steps], dt_f32, tag="dtneg")
        nc.vector.tensor_scalar_mul(dtneg_sb[:], dt_sb[:], -1.0)
        ht_sb = cpool.tile([128, 128], dt_f16, tag="htsb")
        nc.gpsimd.memset(ht_sb[:], 0.0)
        xz_sb0 = cpool.tile([BL + 1, G], dt_f16, tag="xzsb0")
        xz_sb1 = cpool.tile([BL + 1, G], dt_f16, tag="xzsb1")
        xz_sb = [xz_sb0, xz_sb1]
        for i in range(2):
            nc.sync.dma_start(xz_sb[i][BL:BL + 1, :],
                              wg_sb[0:1, C_BP:C_BP + G])

        # warm the activation table set once (exp+ln live together)
        warm = cpool.tile([1, 8], dt_f32, tag="warm")
        nc.gpsimd.memset(warm[:], 1.0)
        nc.scalar.activation(warm[:], warm[:], AF.Exp)
        nc.scalar.activation(warm[:], warm[:], AF.Ln)

        # initial state
        c_prev = sp_.tile([BL, H], dt_f32, tag="c")
        cb_prev = sp_.tile([BL, H], dt_f32, tag="cb")
        nc.gpsimd.memset(c_prev[:], 0.0)
        nc.gpsimd.memset(cb_prev[:], 0.0)

        # xz scratch in device DRAM, [b_loc, t, gate]
        xz_dram = dpool.tile([BL, t_steps, G], dt_f16, tag="xzd")

        def emit_precompute(m):
            """xz rows m*128:(m+1)*128 (= b_loc m//2, t-range (m%2)*128)."""
            b0, t0 = m // 2, (m % 2) * 128
            x_sb = xtp.tile([128, H], dt_f16, tag="xsb")
            nc.sync.dma_start(x_sb[:], xn[m * 128:(m + 1) * 128, :])
            xt_ps = htpp.tile([128, H], dt_f16, tag="tps")
            for k in range(KT):
                nc.tensor.matmul(
                    xt_ps[:, k * 128:(k + 1) * 128],
                    x_sb[:, k * 128:(k + 1) * 128],
                    wg_sb[:, C_I128:C_I128 + 128],
                    is_transpose=True, start=(k == 0), stop=(k == KT - 1))
            xt_sb = xtp.tile([128, H], dt_f16, tag="xtsb")
            nc.scalar.copy(out=xt_sb[:], in_=xt_ps[:])
            for n in range(7):
                ps = z1p.tile([128, 512], dt_f32, tag="z1")
                for k in range(KT):
                    nc.tensor.matmul(
                        ps[:], xt_sb[:, k * 128:(k + 1) * 128],
                        wg_sb[:, k * G + n * 512:k * G + (n + 1) * 512],
                        start=(k == 0), stop=(k == KT - 1))
                xzt = xsp.tile([128, 512], dt_f16, tag="xzstage")
                nc.scalar.copy(out=xzt[:], in_=ps[:])
                nc.sync.dma_start(
                    xz_dram[b0, t0:t0 + 128, n * 512:(n + 1) * 512], xzt[:])

        def sigmoid_into(dst, u_src):
            """dst = 1/(1+u_src) ; u_src = exp(-z) already computed."""
            nc.vector.tensor_scalar_add(u_src[:], u_src[:], 1.0)
            nc.vector.reciprocal_approx_fast(out=dst[:], in_=u_src[:])

        def emit_step(t, c_prev, cb_prev):
            buf = t % 2
            nbuf = (t + 1) % 2
            if t + 1 < t_steps:
                nc.sync.dma_start(xz_sb[nbuf][0:BL, :],
                                  xz_dram[:, t + 1, :])

            # ---- PE: z = xz_t + b + h @ Wh
            z_fi = z2p.tile([BL, 1024], dt_f32, tag="zfi")
            z_fb = z2p.tile([BL, 1024], dt_f32, tag="zfb")
            z_go = z1p.tile([BL, 512], dt_f32, tag="z1")
            z_gz = z1p.tile([BL, 512], dt_f32, tag="z1")
            z_gd = z1p.tile([BL, 512], dt_f32, tag="z1")
            banks = [(z_fi, 0, BK["gf"]), (z_fi, 512, BK["gi"]),
                     (z_gz, 0, BK["gz"]), (z_gd, 0, BK["gd"]),
                     (z_fb, 0, BK["gfb"]), (z_fb, 512, BK["gib"]),
                     (z_go, 0, BK["go"])]
            for (zt, off, bk) in banks:
                dst = zt[:, off:off + 512]
                nc.tensor.matmul(
                    dst, wg_sb[0:BL + 1, C_I33:C_I33 + BL],
                    xz_sb[buf][:, bk * 512:(bk + 1) * 512],
                    start=True, stop=False)
                for k in range(KT):
                    nc.tensor.matmul(
                        dst, ht_sb[:, k * BL:(k + 1) * BL],
                        wg_sb[:, KTG + k * G + bk * 512:
                              KTG + k * G + (bk + 1) * 512],
                        start=False, stop=(k == KT - 1))

            # ---- gates
            u_fi = gp.tile([BL, 1024], dt_f32, tag="ufi")
            nc.scalar.activation(u_fi[:], z_fi[:], AF.Exp, scale=-1.0)
            s_fi = gp.tile([BL, 1024], dt_f32, tag="sfi")
            sigmoid_into(s_fi, u_fi)                       # gf | gi

            u_gz = gp.tile([BL, 512], dt_f32, tag="ugz")
            nc.scalar.activation(u_gz[:], z_gz[:], AF.Exp, scale=-2.0)
            nc.vector.tensor_scalar_add(u_gz[:], u_gz[:], 1.0)
            t_z = gp.tile([BL, 512], dt_f32, tag="tz")
            nc.vector.reciprocal_approx_fast(out=t_z[:], in_=u_gz[:])
            nc.vector.tensor_scalar(t_z[:], t_z[:], -2.0, 1.0,
                                    OP.mult, OP.add)       # tanh(gz)

            # decay: E = exp(-dt * softplus(zd))
            u_gd = gp.tile([BL, 512], dt_f32, tag="ugd")
            nc.scalar.activation(u_gd[:], z_gd[:], AF.Exp)
            nc.gpsimd.tensor_scalar_add(u_gd[:], u_gd[:], 1.0)
            sp_t = gp.tile([BL, 512], dt_f32, tag="spt")
            nc.scalar.activation(sp_t[:], u_gd[:], AF.Ln)
            e_t = gp.tile([BL, 512], dt_f32, tag="et")
            nc.scalar.activation(e_t[:], sp_t[:], AF.Exp,
                                 scale=dtneg_sb[:, t:t + 1])

            u_fb = gp.tile([BL, 1024], dt_f32, tag="ufb")
            nc.scalar.activation(u_fb[:], z_fb[:], AF.Exp, scale=-1.0)
            s_fb = gp.tile([BL, 1024], dt_f32, tag="sfb")
            sigmoid_into(s_fb, u_fb)                       # gfb | gib

            u_go = gp.tile([BL, 512], dt_f32, tag="ugo")
            nc.scalar.activation(u_go[:], z_go[:], AF.Exp, scale=-1.0)
            nc.gpsimd.tensor_scalar_add(u_go[:], u_go[:], 1.0)
            s_go = gp.tile([BL, 512], dt_f32, tag="sgo")
            nc.vector.reciprocal_approx_fast(out=s_go[:], in_=u_go[:])

            # ---- state update
            p1 = gp.tile([BL, 512], dt_f32, tag="p1")
            nc.vector.tensor_mul(p1[:], s_fi[:, 0:512], c_prev[:])
            p2 = gp.tile([BL, 512], dt_f32, tag="p2")
            nc.gpsimd.tensor_mul(p2[:], s_fi[:, 512:1024], t_z[:])
            c_new = gp.tile([BL, H], dt_f32, tag="cn")
            nc.vector.tensor_add(c_new[:], p1[:], p2[:])

            q1 = gp.tile([BL, 512], dt_f32, tag="q1")
            nc.vector.tensor_mul(q1[:], s_fb[:, 0:512], cb_prev[:])
            q2 = gp.tile([BL, 512], dt_f32, tag="q2")
            nc.gpsimd.tensor_mul(q2[:], s_fb[:, 512:1024], t_z[:])
            cb_new = sp_.tile([BL, H], dt_f32, tag="cb")
            nc.vector.tensor_add(cb_new[:], q1[:], q2[:])

            w_t = gp.tile([BL, 512], dt_f32, tag="wt")
            nc.vector.tensor_sub(w_t[:], c_new[:], cb_new[:])
            nc.vector.tensor_mul(w_t[:], w_t[:], e_t[:])
            cd = sp_.tile([BL, H], dt_f32, tag="c")   # carried decayed cell
            nc.vector.tensor_add(cd[:], cb_new[:], w_t[:])

            # h = go * tanh(cd)
            u_c = gp.tile([BL, 512], dt_f32, tag="uc")
            nc.scalar.activation(u_c[:], cd[:], AF.Exp, scale=-2.0)
            nc.vector.tensor_scalar_add(u_c[:], u_c[:], 1.0)
            t_c = gp.tile([BL, 512], dt_f32, tag="tc")
            nc.vector.reciprocal_approx_fast(out=t_c[:], in_=u_c[:])
            nc.vector.tensor_scalar(t_c[:], t_c[:], -2.0, 1.0,
                                    OP.mult, OP.add)
            h16 = hp.tile([BL, H], dt_f16, tag="h16")
            nc.vector.tensor_mul(h16[:], s_go[:], t_c[:])
            oi = hp.tile([BL, H], dt_i8, tag="oi")
            nc.vector.tensor_scalar_mul(oi[:], h16[:], 127.0)
            nc.sync.dma_start(out[t], oi[:])

            # hT for next step: 4 PE transposes, one group, then one copy
            if t + 1 < t_steps:
                htp = htpp.tile([128, H], dt_f16, tag="tps")
                for k in range(KT):
                    nc.tensor.matmul(
                        htp[:, k * BL:(k + 1) * BL],
                        h16[:, k * 128:(k + 1) * 128],
                        wg_sb[0:BL, C_I32:C_I32 + BL],
                        is_transpose=True,
                        start=(k == 0), stop=(k == KT - 1))
                nc.scalar.copy(out=ht_sb[:], in_=htp[:, 0:128])

            return cd, cb_new

        # xz precompute: even M-tiles cover t<128 for every b, odd the rest.
        for m in range(0, mt, 2):
            emit_precompute(m)
        for m in range(1, mt, 2):
            emit_precompute(m)
        nc.sync.dma_start(xz_sb[0][0:BL, :], xz_dram[:, 0, :])
        for t in range(t_steps):
            c_prev, cb_prev = emit_step(t, c_prev, cb_prev)

    nc.compile()
    return nc


def _get_rt(t_steps=T):
    """Build-once runtime: bass module + cached jit callables."""
    key = ("rt", t_steps)
    if key in _CACHE:
        return _CACHE[key]
    import jax
    import jax.numpy as jnp
    import concourse.mybir as mybir
    from concourse import bass2jax
    from concourse.bass2jax import shard_map, Mesh, PartitionSpec
    from jax.sharding import NamedSharding
    from types import SimpleNamespace

    bass2jax.install_neuronx_cc_hook()
    nc = _build(t_steps)

    partition_name = (nc.partition_id_tensor.name
                      if nc.partition_id_tensor else None)
    in_names, out_names, out_avals = [], [], []
    for alloc in nc.m.functions[0].allocations:
        if not isinstance(alloc, mybir.MemoryLocationSet):
            continue
        name = alloc.memorylocations[0].name
        if alloc.kind == "ExternalInput":
            if name != partition_name:
                in_names.append(name)
        elif alloc.kind == "ExternalOutput":
            out_names.append(name)
            shape = tuple(alloc.tensor_shape)
            out_avals.append(
                jax.core.ShapedArray(shape, mybir.dt.np(alloc.dtype)))
    n_params = len(in_names)
    all_names = in_names + out_names
    if partition_name is not None:
        all_names = all_names + [partition_name]
    all_names = tuple(all_names)
    donate = tuple(range(n_params, n_params + len(out_names)))

    def _body(*args):
        operands = list(args)
        if partition_name is not None:
            operands.append(bass2jax.partition_id_tensor())
        outs = bass2jax._bass_exec_p.bind(
            *operands, out_avals=tuple(out_avals), in_names=all_names,
            out_names=tuple(out_names), lowering_input_output_aliases=(),
            sim_require_finite=True, sim_require_nnan=True, nc=nc)
        return tuple(outs)

    devs = jax.devices()[:NCORES]
    mesh = Mesh(np.asarray(devs), ("core",))
    P = PartitionSpec
    # out assembled as [T, NCORES*BL, H] == [T, B, H]: no host transpose.
    # The donated zero buffers use the same sharding so aliasing is clean.
    in_specs = (P("core"),) * n_params + (P(None, "core"),) * len(out_names)
    sharded = jax.jit(
        shard_map(_body, mesh=mesh, in_specs=in_specs,
                  out_specs=(P(None, "core"),) * len(out_names),
                  check_rep=False),
        donate_argnums=donate, keep_unused=True)

    def _prep_body(w):
        wr = jax.lax.all_gather(w, "core", axis=0, tiled=True)
        z = jnp.zeros((t_steps, BL, H), jnp.int8)
        return wr, z

    prep = jax.jit(
        shard_map(_prep_body, mesh=mesh, in_specs=(P("core"),),
                  out_specs=(P("core"), P(None, "core")), check_rep=False))

    rt = SimpleNamespace(
        nc=nc, sharded=sharded, prep=prep, in_names=in_names,
        shc=NamedSharding(mesh, P("core")), jax=jax)
    _CACHE[key] = rt
    return rt


def _pack_wg(W, b):
    Wp = np.asarray(W, np.float32)[:, PERM].astype(F16)
    wx, wh = Wp[:H], Wp[H:]
    wg = np.zeros((128, CW), F16)
    for k in range(KT):
        wg[:, k * G:(k + 1) * G] = wx[k * 128:(k + 1) * 128]
        wg[:, KTG + k * G:KTG + (k + 1) * G] = wh[k * 128:(k + 1) * 128]
    wg[0:BL, C_I33:C_I33 + BL] = np.eye(BL, dtype=F16)
    wg[BL, C_I33:C_I33 + BL] = 1.0                      # bias inject row
    wg[0:BL, C_I32:C_I32 + BL] = np.eye(BL, dtype=F16)
    wg[0:128, C_I128:C_I128 + 128] = np.eye(128, dtype=F16)
    wg[0, C_BP:C_BP + G] = np.asarray(b, np.float32)[PERM].astype(F16)
    return wg


def kernel(x, dt, W, b):
    import os
    import time
    dbg = os.environ.get("KERNEL_DEBUG_TIMING")
    t0 = time.time()
    rt = _get_rt(T)
    jax = rt.jax

    wg = _pack_wg(W, b)
    dtg = np.ascontiguousarray(np.asarray(dt, np.float32))
    xg = np.asarray(x, np.float32).reshape(B * T, H).astype(F16)
    t1 = time.time()

    # wg/dtg first: prep's all_gather+zeros overlap the big x upload
    wd, dtd, xd = jax.device_put((wg, dtg, xg), (rt.shc, rt.shc, rt.shc))
    wr, zz = rt.prep(wd)
    args = {"xn": xd, "wg": wr, "dtr": dtd}
    (outg,) = rt.sharded(*[args[n] for n in rt.in_names], zz)
    t2 = time.time()
    # shard-wise fetch with fused int8->f32 decode: the per-shard decode
    # interleaves into I/O-wait windows instead of contending after the
    # full fetch (1 vCPU; the tunnel client burns CPU post-transfer)
    res = np.empty((T, B, H), np.float32)
    outg.copy_to_host_async()
    for sh in outg.addressable_shards:
        np.multiply(np.asarray(sh.data), np.float32(1.0 / 127.0),
                    out=res[sh.index], dtype=np.float32)
    if dbg:
        print(f"  [kernel] hostprep={t1-t0:.2f} dispatch={t2-t1:.2f} "
              f"fetch+decode={time.time()-t2:.2f}", flush=True)
    return res
